# revision 5
# baseline (speedup 1.0000x reference)
"""Trainium2 Bass kernel for masked GAT-style attention softmax.

reference: softmax(where(mask, -1e9, leakyrelu(s1[:,None]+s2[None,:])), -1)
with s1 = x@w1, s2 = x@w2.  B=8 batches -> data-parallel over 8 NeuronCores.

Per-core layout [i_part, j_free], fp16 compute / f32 accum:
  PE  : s1, s2 projections; broadcasts (rank-1 matmuls)
  DVE : mask combine  w = -100*m + s2b   (scalar_tensor_tensor, u8 in)
        (some tiles)  leakyrelu via  y = w+s1 ;  lr = max(.2y, y)
        normalize     out = p * (1/r)    (tensor_scalar, per-part scalar)
  ACT : (most tiles)  lr = Prelu(w + s1[i], alpha=.2)   [same table set as exp]
        p = Exp(lr - c[i]),  accum_out -> rowsum r      [c = row max bound]
"""

import numpy as np

B, N, F = 8, 4096, 256
P = 128
NT = N // P  # 32 row tiles per core
MASKC = -100.0
ALPHA = 0.2

# fraction of row-tiles whose leakyrelu runs on ScalarE (rest on VectorE)
N_ACT_TILES = 25


def build(n_act=N_ACT_TILES, out_dt_name="float16"):
    from contextlib import ExitStack

    import concourse.bass as bass  # noqa: F401
    import concourse.mybir as mybir
    import concourse.tile as tile
    from concourse import bacc

    dt = mybir.dt
    Alu = mybir.AluOpType
    Act = mybir.ActivationFunctionType
    cdt = dt.float16
    odt = getattr(dt, out_dt_name)

    nc = bacc.Bacc("TRN2", target_bir_lowering=False, debug=False, num_devices=8)
    xt_ext = nc.dram_tensor("xt", [F, N], dt.float32, kind="ExternalInput").ap()
    m_ext = nc.dram_tensor("mask", [N, N], dt.uint8, kind="ExternalInput").ap()
    w_ext = nc.dram_tensor("w", [F, 2], dt.float32, kind="ExternalInput").ap()
    out_ext = nc.dram_tensor("out", [N, N], odt, kind="ExternalOutput").ap()

    # spread the DVE-leaky tiles evenly among the ACT-leaky ones
    n_dve = NT - n_act
    dve_tiles = {t for t in range(NT) if (t * n_dve) // NT != ((t + 1) * n_dve) // NT}

    with tile.TileContext(nc) as tc, ExitStack() as ctx:
        persist = ctx.enter_context(tc.tile_pool(name="persist", bufs=1))
        psum = ctx.enter_context(tc.tile_pool(name="psum", bufs=2, space="PSUM"))

        s2row = persist.tile([1, N], dt.float32, tag="s2row")
        s1col = persist.tile([P, NT], dt.float32, tag="s1col")
        cneg = persist.tile([P, NT], dt.float32, tag="cneg")
        s2b = persist.tile([P, N], cdt, tag="s2b")
        ones128 = persist.tile([1, P], dt.float32, tag="ones")

        with tc.tile_pool(name="setup", bufs=1) as setup:
            xt_sb = setup.tile([P, 2, N], dt.float32, tag="xt")
            for a in range(2):
                nc.sync.dma_start(xt_sb[:, a, :], xt_ext[a * P : (a + 1) * P, :])
            w_sb = setup.tile([P, 2, 2], dt.float32, tag="w")
            for a in range(2):
                nc.sync.dma_start(w_sb[:, a, :], w_ext[a * P : (a + 1) * P, :])

            # s2row[0,:] = s2 = x @ w2   (row of length N)
            for j in range(N // 512):
                ps = psum.tile([1, 512], dt.float32, tag="ps12")
                for a in range(2):
                    nc.tensor.matmul(
                        ps[:],
                        w_sb[:, a, 1:2],
                        xt_sb[:, a, j * 512 : (j + 1) * 512],
                        start=(a == 0),
                        stop=(a == 1),
                    )
                nc.vector.tensor_copy(s2row[:, j * 512 : (j + 1) * 512], ps[:])

            # s1col[p, t] = s1[t*P + p]  (column layout for per-partition use)
            for t in range(NT):
                ps1 = psum.tile([P, 1], dt.float32, tag="ps1col")
                for a in range(2):
                    nc.tensor.matmul(
                        ps1[:],
                        xt_sb[:, a, t * P : (t + 1) * P],
                        w_sb[:, a, 0:1],
                        start=(a == 0),
                        stop=(a == 1),
                    )
                nc.vector.tensor_copy(s1col[:, t : t + 1], ps1[:])

        nc.vector.memset(ones128[:], 1.0)

        # s2b[p, j] = s2[j]  broadcast across partitions (rank-1 matmul)
        for j in range(N // 512):
            psb = psum.tile([P, 512], dt.float32, tag="psb")
            nc.tensor.matmul(
                psb[:],
                ones128[:],
                s2row[:, j * 512 : (j + 1) * 512],
                start=True,
                stop=True,
            )
            nc.vector.tensor_copy(s2b[:, j * 512 : (j + 1) * 512], psb[:])

        # c[i] = leakyrelu(s1[i] + max(s2)) >= rowmax of lr; exp bias = -c
        s2m1 = persist.tile([1, 1], dt.float32, tag="s2m1")
        nc.vector.tensor_reduce(s2m1[:], s2row[:], mybir.AxisListType.X, Alu.max)
        psm = psum.tile([P, 1], dt.float32, tag="psm")
        nc.tensor.matmul(psm[:], ones128[:], s2m1[:], start=True, stop=True)
        s2m = persist.tile([P, 1], dt.float32, tag="s2m")
        nc.vector.tensor_copy(s2m[:], psm[:])
        ycol = persist.tile([P, NT], dt.float32, tag="ycol")
        nc.vector.tensor_scalar_add(ycol[:], s1col[:], s2m[:, 0:1])
        nc.vector.scalar_tensor_tensor(cneg[:], ycol[:], ALPHA, ycol[:], Alu.mult, Alu.max)
        nc.vector.tensor_scalar_mul(cneg[:], cneg[:], -1.0)

        mp = ctx.enter_context(tc.tile_pool(name="mask", bufs=3))
        wp = ctx.enter_context(tc.tile_pool(name="work", bufs=2))
        pp = ctx.enter_context(tc.tile_pool(name="prob", bufs=2))
        op = ctx.enter_context(tc.tile_pool(name="outp", bufs=3))
        rp = ctx.enter_context(tc.tile_pool(name="redu", bufs=4))

        for t in range(NT):
            m_sb = mp.tile([P, N], dt.uint8, tag="m")
            nc.sync.dma_start(m_sb[:], m_ext[t * P : (t + 1) * P, :])

            # w = -100*m + s2[j]
            w_t = wp.tile([P, N], cdt, tag="wt")
            nc.vector.scalar_tensor_tensor(
                w_t[:], m_sb[:], MASKC, s2b[:], Alu.mult, Alu.add
            )

            lr = wp.tile([P, N], cdt, tag="lr")
            if t in dve_tiles:
                y = wp.tile([P, N], cdt, tag="y")
                nc.vector.tensor_scalar_add(y[:], w_t[:], s1col[:, t : t + 1])
                nc.vector.scalar_tensor_tensor(
                    lr[:], y[:], ALPHA, y[:], Alu.mult, Alu.max
                )
            else:
                nc.scalar.activation(
                    lr[:],
                    w_t[:],
                    Act.Prelu,
                    bias=s1col[:, t : t + 1],
                    scale=1.0,
                    alpha=ALPHA,
                )

            p_t = pp.tile([P, N], cdt, tag="p")
            r_t = rp.tile([P, 1], dt.float32, tag="r")
            nc.scalar.activation(
                p_t[:],
                lr[:],
                Act.Exp,
                bias=cneg[:, t : t + 1],
                scale=1.0,
                accum_out=r_t[:],
            )

            rec = rp.tile([P, 1], dt.float32, tag="rec")
            nc.vector.reciprocal(rec[:], r_t[:])

            o_t = op.tile([P, N], odt, tag="o")
            nc.vector.tensor_scalar_mul(o_t[:], p_t[:], rec[:, 0:1])
            nc.sync.dma_start(out_ext[t * P : (t + 1) * P, :], o_t[:])

    nc.compile()
    return nc


def make_in_maps(x, mask, w1, w2):
    x = np.asarray(x, dtype=np.float32)
    mask_u8 = np.asarray(mask).astype(np.uint8)
    w = np.ascontiguousarray(
        np.stack([np.asarray(w1, np.float32), np.asarray(w2, np.float32)], axis=1)
    )
    in_maps = []
    for b in range(B):
        in_maps.append(
            {
                "xt": np.ascontiguousarray(x[b].T),
                "mask": mask_u8[b],
                "w": w,
            }
        )
    return in_maps


def kernel(x, mask, w1, w2, trace=False, nc=None):
    from concourse.bass_utils import run_bass_kernel_spmd

    if trace:
        _install_ntff_hook()
    if nc is None:
        nc = build()
    in_maps = make_in_maps(x, mask, w1, w2)
    res = run_bass_kernel_spmd(nc, in_maps, core_ids=list(range(B)), trace=trace)
    out = np.stack(
        [np.asarray(res.results[b]["out"]).astype(np.float32) for b in range(B)]
    )
    kernel.last_result = res
    return out


def _install_ntff_hook():
    import sys
    import types

    if "antenv.axon_hooks" in sys.modules:
        return
    from trn_agent_boot.trn_boot import _ntff_profile_via_ctypes

    hook = _ntff_profile_via_ctypes("/opt/axon/libaxon_pjrt.so")
    mod = types.ModuleType("antenv.axon_hooks")
    mod.get_axon_ntff_profile_hook = lambda: hook
    mod.set_axon_ntff_profile_hook = lambda h: None
    sys.modules["antenv.axon_hooks"] = mod
    import antenv

    antenv.axon_hooks = mod
